# revision 25
# baseline (speedup 1.0000x reference)
"""Trainium2 Bass kernel for BatchGATConv (GAT message passing).

Strategy (8 NeuronCores, SPMD, dst-partitioned):
  - Host: in-degree-sort each core's 2500 dst nodes into 20 tiles of 128 so
    every tile has near-uniform degree; chunk k of a tile holds the k-th
    in-edge of each of the tile's 128 nodes (slot-aligned, so dst-local ==
    partition and no one-hot matmuls are needed). Padded slots point at a
    dedicated pad row whose attention logit is -150 (=> exp == 0).
  - Projection (replicated on all cores, bf16): g[pos] row pair
    [ft(b0)|el|er | ft(b1)|el|er | pad] with attn_l/attn_r folded into the
    weight matrix on the host; 1024-row blocks, batched DMAs.
  - Aggregation per tile: dma_gather fetches the (K+1)*128 row pairs in
    <=1024-index calls (SWDGE descriptor-ring cap) rotated over 4 SWDGE
    queues (the +1 chunk gathers the tile's own rows for er_dst); logits
    lg = el_src + er_dst, leaky, exp on ACT; messages ft*ex scaled in-place
    on DVE; PSUM-accumulated via identity matmuls (per-partition segment sum);
    denominator via DVE reduce; out = leaky(num/den).
"""

import numpy as np

try:
    import concourse.bass as bass
except ImportError:  # pragma: no cover
    import sys

    sys.path.insert(0, "/opt/trn_rl_repo")
    import concourse.bass as bass

import ml_dtypes
import concourse.bacc as bacc
import concourse.mybir as mybir
import concourse.tile as tile
from concourse.bass_utils import run_bass_kernel_spmd

P = 128
F32 = mybir.dt.float32
BF16 = mybir.dt.bfloat16
I32 = mybir.dt.int32
I16 = mybir.dt.int16

# problem constants
N, B, DIN, H, D, E = 20000, 2, 128, 4, 64, 320000
NEG = 0.2
NCORES = 8
NPC = N // NCORES  # 2500 dst nodes per core
NT = 20  # node tiles per core
NPOSC = NT * P  # 2560 positions per core (60 pad positions)
NPOS = NCORES * NPOSC  # 20480 global positions
PADROW = NPOS  # pad pair row index
GROWS = NPOS + 2
WC = 264  # per-(n,b) block: [ft 256 | el 4 | er 4]
HD = H * D  # 256
FT = B * HD  # 512
RW = 640  # g row width (2*WC + 112 pad; 1280B, %256 for dma_gather)
PAD_EL = -150.0
QBLK = 2048  # projection rows per block
NBLK = NPOS // QBLK  # 10 per batch
NSWQ = 4  # SWDGE queues (Q7 descriptor-gen parallelism)


def _host_prep(src, dst):
    """Degree-sort nodes per core; build per-core wrapped int16 gather tables.

    Returns (K, idxs_all, pos2node):
      K: per-tile chunk count, shared across cores (max in-degree in tile)
      idxs_all[c]: [16, sum((K+1)*8)] int16 wrapped gather indices
      pos2node: [NPOS] original node id per position (-1 = pad)
    """
    deg = np.bincount(dst, minlength=N).astype(np.int64)
    pos2node = np.full(NPOS, -1, np.int64)
    for c in range(NCORES):
        ids = np.arange(c * NPC, (c + 1) * NPC)
        order = np.argsort(-deg[ids], kind="stable")
        pos2node[c * NPOSC : c * NPOSC + NPC] = ids[order]
    node2pos = np.empty(N, np.int64)
    real = pos2node >= 0
    node2pos[pos2node[real]] = np.nonzero(real)[0]

    first = np.arange(NCORES)[:, None] * NPOSC + np.arange(NT)[None, :] * P
    K = np.maximum(deg[pos2node[first]].max(axis=0), 1)  # [NT]

    dpos = node2pos[dst]
    spos = node2pos[src]
    order_e = np.argsort(dpos, kind="stable")
    ds = dpos[order_e]
    ss = spos[order_e]
    starts = np.searchsorted(ds, np.arange(NPOS + 1))
    k_e = np.arange(E) - starts[ds]

    idxs_all = []
    for c in range(NCORES):
        cols = []
        for t in range(NT):
            Kt = int(K[t])
            base = c * NPOSC + t * P
            sl = np.full((Kt + 1) * P, PADROW, np.int64)
            e0, e1 = starts[base], starts[base + P]
            pp_ = ds[e0:e1] - base
            sl[k_e[e0:e1] * P + pp_] = ss[e0:e1]
            sl[Kt * P : (Kt + 1) * P] = base + np.arange(P)
            cols.append(sl.reshape(-1, 16).T)
        idxs_all.append(
            np.ascontiguousarray(
                np.tile(np.concatenate(cols, axis=1), (8, 1))
            ).astype(np.int16)
        )
    return list(map(int, K)), idxs_all, pos2node


def _build(K, stage='all', lrelu=False):
    S8 = sum((k + 1) * 8 for k in K)
    KP1M = max(K) + 1
    AF = mybir.ActivationFunctionType
    OP = mybir.AluOpType

    nc = bacc.Bacc(trn_type="TRN2", num_swdge_queues=NSWQ)
    featT = nc.dram_tensor("featT", [DIN, B * NPOS], BF16, kind="ExternalInput")
    wmat = nc.dram_tensor("wmat", [DIN, WC], BF16, kind="ExternalInput")
    idxsd = nc.dram_tensor("idxs", [128, S8], I16, kind="ExternalInput")
    outd = nc.dram_tensor("out", [NPOSC, FT], F32, kind="ExternalOutput")
    g = nc.dram_tensor("gtab", [GROWS, RW], BF16)

    with tile.TileContext(nc) as tc:
        with (
            tc.tile_pool(name="const", bufs=1) as cp,
            tc.tile_pool(name="proj", bufs=3) as pp,
            tc.tile_pool(name="projps", bufs=5, space="PSUM") as ppp,
            tc.tile_pool(name="agg", bufs=2) as ag,
            tc.tile_pool(name="small", bufs=4) as sm,
            tc.tile_pool(name="accps", bufs=3, space="PSUM") as psp,
            tc.tile_pool(name="exb", bufs=2) as eb,
        ):
            # resident constants
            w_sb = cp.tile([DIN, WC], BF16)
            nc.sync.dma_start(w_sb[:], wmat[:])
            idxs_res = cp.tile([128, S8], I16)
            nc.sync.dma_start(idxs_res[:], idxsd[:])
            iota_r = cp.tile([P, P], I32)
            nc.gpsimd.iota(iota_r[:], pattern=[[1, P]], base=0, channel_multiplier=0)
            iota_rf = cp.tile([P, P], F32)
            nc.vector.tensor_copy(iota_rf[:], iota_r[:])
            iota_p = cp.tile([P, 1], I32)
            nc.gpsimd.iota(iota_p[:], pattern=[[1, 1]], base=0, channel_multiplier=1)
            iota_pf = cp.tile([P, 1], F32)
            nc.vector.tensor_copy(iota_pf[:], iota_p[:])
            ident = cp.tile([P, P], BF16)
            nc.vector.tensor_scalar(
                out=ident[:],
                in0=iota_rf[:],
                scalar1=iota_pf[:, 0:1],
                scalar2=None,
                op0=OP.is_equal,
            )
            # pad row: ft/er = 0, el = PAD_EL -> exp(leaky(lg)) == 0
            padt = cp.tile([1, RW], BF16)
            nc.gpsimd.memset(padt[:], 0.0)
            nc.gpsimd.memset(padt[:, 256:260], PAD_EL)
            nc.gpsimd.memset(padt[:, WC + 256 : WC + 260], PAD_EL)
            nc.sync.dma_start(out=g[NPOS : NPOS + 1, :], in_=padt[:])

            # ---- projection: g[q] = [ft|el|er](b0) | [ft|el|er](b1) ----
            hw = (nc.sync, nc.scalar)
            for b in range(B if stage in ('all', 'proj') else 0):
                for blk in range(NBLK):
                    j = b * NBLK + blk
                    ftb = pp.tile([DIN, QBLK], BF16, tag="ftb")
                    hw[j % 2].dma_start(
                        ftb[:],
                        featT[:, b * NPOS + blk * QBLK : b * NPOS + (blk + 1) * QBLK],
                    )
                    pw = pp.tile([P, QBLK // P, WC], BF16, tag="pw")
                    for i in range(QBLK // P):
                        po = ppp.tile([P, WC], F32, tag="po")
                        nc.tensor.matmul(
                            po[:],
                            lhsT=ftb[:, i * P : (i + 1) * P],
                            rhs=w_sb[:],
                            start=True,
                            stop=True,
                        )
                        if i % 2 == 0:
                            nc.vector.tensor_copy(pw[:, i, :], po[:])
                        else:
                            nc.scalar.activation(pw[:, i, :], po[:], AF.Copy)
                    dst_ap = g[
                        blk * QBLK : (blk + 1) * QBLK, b * WC : (b + 1) * WC
                    ].rearrange("(i p) c -> p i c", p=P)
                    hw[(j + 1) % 2].dma_start(out=dst_ap, in_=pw[:])

            # ---- aggregation: per tile, slot-aligned segment softmax+sum ----
            off8 = 0
            ncall = 0
            pending = []

            def _finalize(t, acc, ex, Kt):
                dsum = sm.tile([P, B * H], F32, tag="dsum")
                nc.vector.tensor_reduce(
                    out=dsum[:],
                    in_=ex[:, 0:Kt, :].rearrange("p k q -> p q k"),
                    axis=mybir.AxisListType.X,
                    op=OP.add,
                )
                dse = sm.tile([P, B * H], F32, tag="dse")
                nc.vector.tensor_scalar_add(dse[:], dsum[:], 1e-20)
                rcp = sm.tile([P, B * H, 1], F32, tag="rcp")
                nc.vector.reciprocal(rcp[:, :, 0], dse[:])
                o1 = sm.tile([P, B * H, D], F32, tag="o1")
                nc.vector.tensor_tensor(
                    out=o1[:],
                    in0=acc[:].rearrange("p (q d) -> p q d", d=D),
                    in1=rcp[:].to_broadcast([P, B * H, D]),
                    op=OP.mult,
                )
                og = sm.tile([P, FT], F32, tag="og")
                nc.vector.scalar_tensor_tensor(
                    out=og[:],
                    in0=o1[:].rearrange("p q d -> p (q d)"),
                    scalar=NEG,
                    in1=o1[:].rearrange("p q d -> p (q d)"),
                    op0=OP.mult,
                    op1=OP.max,
                )
                nc.sync.dma_start(out=outd[t * P : (t + 1) * P, :], in_=og[:])

            for t in range(NT if stage in ('all', 'agg') else 0):
                Kt = K[t]
                KP = Kt + 1
                gt = ag.tile([P, KP1M, RW], BF16, tag="gt")
                # SWDGE descriptor ring holds ~1024 pairs: cap 8 chunks/call;
                # rotate the SWDGE queues to overlap Q7 descriptor gen
                for g0 in range(0, KP, 8):
                    gn = min(8, KP - g0)
                    nc.gpsimd.dma_gather(
                        out_ap=gt[:, g0 : g0 + gn, :],
                        in_ap=g[:],
                        idxs_ap=idxs_res[:, off8 + g0 * 8 : off8 + (g0 + gn) * 8],
                        num_idxs=gn * P,
                        num_idxs_reg=gn * P,
                        elem_size=RW,
                        queue_num=ncall % NSWQ,
                    )
                    ncall += 1
                off8 += KP * 8

                # logits lg[p, k, (b,h)] = el_src + er_own (k-major: all the
                # downstream DVE/ACT ops stream contiguously)
                lg = sm.tile([P, KP1M, B * H], BF16, tag="lg")
                elv = gt[:, 0:Kt, 0 : 2 * WC].rearrange(
                    "p k (b r) -> p k b r", r=WC
                )[:, :, :, HD : HD + H]
                erv = gt[:, Kt, 0 : 2 * WC].rearrange("p (b r) -> p b r", r=WC)[
                    :, :, HD + H : HD + 2 * H
                ]
                lgv = lg[:, 0:Kt, :].rearrange("p k (b h) -> p k b h", h=H)
                nc.vector.tensor_tensor(
                    out=lgv,
                    in0=elv,
                    in1=erv[:, None, :, :].to_broadcast([P, Kt, B, H]),
                    op=OP.add,
                )
                l1 = sm.tile([P, KP1M, B * H], BF16, tag="l1")
                nc.vector.scalar_tensor_tensor(
                    out=l1[:, 0:Kt, :],
                    in0=lg[:, 0:Kt, :],
                    scalar=NEG,
                    in1=lg[:, 0:Kt, :],
                    op0=OP.mult,
                    op1=OP.max,
                )
                ex = sm.tile([P, KP1M, B * H], BF16, tag="ex")
                nc.scalar.activation(ex[:, 0:Kt, :], l1[:, 0:Kt, :], AF.Exp)
                # materialize b1's exponents broadcast on ACT so that half of
                # the multiply has unit-stride operands (DVE 2x bf16 mode)
                exb = eb.tile([P, KP1M, HD], BF16, tag="exb")
                nc.scalar.activation(
                    exb[:, 0:Kt, :].rearrange("p k (h d) -> p k h d", d=D),
                    ex[:, 0:Kt, H : B * H][:, :, :, None].to_broadcast(
                        [P, Kt, H, D]
                    ),
                    AF.Copy,
                )

                # messages: ft *= ex (in-place, per batch)
                for b in range(B):
                    ftv = gt[:, 0:Kt, b * WC : b * WC + HD].rearrange(
                        "p k (h d) -> p k h d", d=D
                    )
                    if b == 1:
                        in1 = exb[:, 0:Kt, :].rearrange(
                            "p k (h d) -> p k h d", d=D
                        )
                    else:
                        exv = ex[:, 0:Kt, b * H : (b + 1) * H]
                        in1 = exv[:, :, :, None].to_broadcast([P, Kt, H, D])
                    nc.vector.tensor_tensor(
                        out=ftv,
                        in0=ftv,
                        in1=in1,
                        op=OP.mult,
                    )

                # segment sum via identity matmuls accumulating in PSUM
                acc = psp.tile([P, FT], F32, tag="acc")
                for k in range(Kt):
                    rhs = gt[:, k, 0 : 2 * WC].rearrange("p (b r) -> p b r", r=WC)[
                        :, :, 0:HD
                    ]
                    nc.tensor.matmul(
                        acc[:],
                        lhsT=ident[:],
                        rhs=rhs,
                        start=(k == 0),
                        stop=(k == Kt - 1),
                    )

                pending.append((t, acc, ex, Kt))
                if len(pending) > 2:
                    _finalize(*pending.pop(0))
            for args in pending:
                _finalize(*args)

    nc.compile()
    return nc


def _make_inputs(feat, W, attn_l, attn_r, src, dst, n_nodes=N, n_cores=NCORES):
    feat = np.asarray(feat, dtype=np.float32)
    W = np.asarray(W, dtype=np.float32)
    attn_l = np.asarray(attn_l, dtype=np.float32)
    attn_r = np.asarray(attn_r, dtype=np.float32)
    src = np.asarray(src)
    dst = np.asarray(dst)

    K, idxs_all, pos2node = _host_prep(src, dst)

    real = pos2node >= 0
    ftp = np.zeros((B, NPOS, DIN), np.float32)
    ftp[:, real, :] = feat[pos2node[real]].transpose(1, 0, 2)
    featT = np.ascontiguousarray(ftp.reshape(B * NPOS, DIN).T).astype(
        ml_dtypes.bfloat16
    )
    Wl = (W.reshape(DIN, H, D) * attn_l[None]).sum(-1)
    Wr = (W.reshape(DIN, H, D) * attn_r[None]).sum(-1)
    wmat = np.concatenate([W, Wl, Wr], axis=1).astype(ml_dtypes.bfloat16)

    in_maps = [
        {"featT": featT, "wmat": wmat, "idxs": idxs_all[c]} for c in range(n_cores)
    ]
    return K, in_maps, pos2node


_CACHE = {}


def kernel(feat, W, attn_l, attn_r, src, dst):
    K, in_maps, pos2node = _make_inputs(feat, W, attn_l, attn_r, src, dst)
    key = tuple(K)
    if key not in _CACHE:
        _CACHE[key] = _build(K)
    nc = _CACHE[key]
    res = run_bass_kernel_spmd(nc, in_maps, list(range(NCORES))).results
    out = np.empty((N, B, H, D), np.float32)
    for c in range(NCORES):
        nodes = pos2node[c * NPOSC : c * NPOSC + NPC]
        out[nodes] = res[c]["out"][:NPC].reshape(NPC, B, H, D)
    return out


if __name__ == "__main__":
    rng = np.random.default_rng(0)
    feat = rng.standard_normal((N, B, DIN), dtype=np.float32)
    W = rng.standard_normal((DIN, H * D), dtype=np.float32) / np.sqrt(DIN)
    al = rng.standard_normal((H, D), dtype=np.float32) * 0.1
    ar = rng.standard_normal((H, D), dtype=np.float32) * 0.1
    src = rng.integers(0, N, E).astype(np.int32)
    dst = rng.integers(0, N, E).astype(np.int32)
    out = kernel(feat=feat, W=W, attn_l=al, attn_r=ar, src=src, dst=dst)
    print(out.shape, out.dtype, np.abs(out).mean())


# revision 26
# speedup vs baseline: 1.0807x; 1.0807x over previous
"""Trainium2 Bass kernel for BatchGATConv (GAT message passing).

Strategy (8 NeuronCores, SPMD, dst-partitioned):
  - Host: in-degree-sort each core's 2500 dst nodes into 20 tiles of 128 so
    every tile has near-uniform degree; chunk k of a tile holds the k-th
    in-edge of each of the tile's 128 nodes (slot-aligned, so dst-local ==
    partition and no one-hot matmuls are needed). Padded slots point at a
    dedicated pad row whose attention logit is -150 (=> exp == 0).
  - Projection (replicated on all cores, bf16): g[pos] row pair
    [ft(b0)|el|er | ft(b1)|el|er | pad] with attn_l/attn_r folded into the
    weight matrix on the host; 1024-row blocks, batched DMAs.
  - Aggregation per tile: dma_gather fetches the (K+1)*128 row pairs in
    <=1024-index calls (SWDGE descriptor-ring cap) rotated over 4 SWDGE
    queues (the +1 chunk gathers the tile's own rows for er_dst); logits
    lg = el_src + er_dst, leaky, exp on ACT; messages ft*ex scaled in-place
    on DVE; PSUM-accumulated via identity matmuls (per-partition segment sum);
    denominator via DVE reduce; out = leaky(num/den).
"""

import numpy as np

try:
    import concourse.bass as bass
except ImportError:  # pragma: no cover
    import sys

    sys.path.insert(0, "/opt/trn_rl_repo")
    import concourse.bass as bass

import ml_dtypes
import concourse.bacc as bacc
import concourse.mybir as mybir
import concourse.tile as tile
from concourse.bass_utils import run_bass_kernel_spmd

P = 128
F32 = mybir.dt.float32
BF16 = mybir.dt.bfloat16
I32 = mybir.dt.int32
I16 = mybir.dt.int16

# problem constants
N, B, DIN, H, D, E = 20000, 2, 128, 4, 64, 320000
NEG = 0.2
NCORES = 8
NPC = N // NCORES  # 2500 dst nodes per core
NT = 20  # node tiles per core
NPOSC = NT * P  # 2560 positions per core (60 pad positions)
NPOS = NCORES * NPOSC  # 20480 global positions
PADROW = NPOS  # pad pair row index
GROWS = NPOS + 2
WC = 264  # per-(n,b) block: [ft 256 | el 4 | er 4]
HD = H * D  # 256
FT = B * HD  # 512
RW = 640  # g row width (2*WC + 112 pad; 1280B, %256 for dma_gather)
PAD_EL = -150.0
QBLK = 2048  # projection rows per block
NBLK = NPOS // QBLK  # 10 per batch
NSWQ = 4  # SWDGE queues (Q7 descriptor-gen parallelism)


def _host_prep(src, dst):
    """Degree-sort nodes per core; build per-core wrapped int16 gather tables.

    Returns (K, idxs_all, pos2node):
      K: per-tile chunk count, shared across cores (max in-degree in tile)
      idxs_all[c]: [16, sum((K+1)*8)] int16 wrapped gather indices
      pos2node: [NPOS] original node id per position (-1 = pad)
    """
    deg = np.bincount(dst, minlength=N).astype(np.int64)
    pos2node = np.full(NPOS, -1, np.int64)
    for c in range(NCORES):
        ids = np.arange(c * NPC, (c + 1) * NPC)
        order = np.argsort(-deg[ids], kind="stable")
        pos2node[c * NPOSC : c * NPOSC + NPC] = ids[order]
    node2pos = np.empty(N, np.int64)
    real = pos2node >= 0
    node2pos[pos2node[real]] = np.nonzero(real)[0]

    first = np.arange(NCORES)[:, None] * NPOSC + np.arange(NT)[None, :] * P
    K = np.maximum(deg[pos2node[first]].max(axis=0), 1)  # [NT]

    dpos = node2pos[dst]
    spos = node2pos[src]
    order_e = np.argsort(dpos, kind="stable")
    ds = dpos[order_e]
    ss = spos[order_e]
    starts = np.searchsorted(ds, np.arange(NPOS + 1))
    k_e = np.arange(E) - starts[ds]

    idxs_all = []
    for c in range(NCORES):
        cols = []
        for t in range(NT):
            Kt = int(K[t])
            base = c * NPOSC + t * P
            sl = np.full((Kt + 1) * P, PADROW, np.int64)
            e0, e1 = starts[base], starts[base + P]
            pp_ = ds[e0:e1] - base
            sl[k_e[e0:e1] * P + pp_] = ss[e0:e1]
            sl[Kt * P : (Kt + 1) * P] = base + np.arange(P)
            cols.append(sl.reshape(-1, 16).T)
        idxs_all.append(
            np.ascontiguousarray(
                np.tile(np.concatenate(cols, axis=1), (8, 1))
            ).astype(np.int16)
        )
    return list(map(int, K)), idxs_all, pos2node


def _build(K, stage='all', lrelu=False):
    S8 = sum((k + 1) * 8 for k in K)
    KP1M = max(K) + 1
    AF = mybir.ActivationFunctionType
    OP = mybir.AluOpType

    nc = bacc.Bacc(trn_type="TRN2", num_swdge_queues=NSWQ)
    featT = nc.dram_tensor("featT", [DIN, B * NPOS], BF16, kind="ExternalInput")
    wmat = nc.dram_tensor("wmat", [DIN, WC], BF16, kind="ExternalInput")
    idxsd = nc.dram_tensor("idxs", [128, S8], I16, kind="ExternalInput")
    outd = nc.dram_tensor("out", [NPOSC, FT], F32, kind="ExternalOutput")
    g = nc.dram_tensor("gtab", [GROWS, RW], BF16)

    with tile.TileContext(nc) as tc:
        with (
            tc.tile_pool(name="const", bufs=1) as cp,
            tc.tile_pool(name="proj", bufs=3) as pp,
            tc.tile_pool(name="projps", bufs=5, space="PSUM") as ppp,
            tc.tile_pool(name="agg", bufs=3) as ag,
            tc.tile_pool(name="small", bufs=4) as sm,
            tc.tile_pool(name="accps", bufs=3, space="PSUM") as psp,
        ):
            # resident constants
            w_sb = cp.tile([DIN, WC], BF16)
            nc.sync.dma_start(w_sb[:], wmat[:])
            idxs_res = cp.tile([128, S8], I16)
            nc.sync.dma_start(idxs_res[:], idxsd[:])
            iota_r = cp.tile([P, P], I32)
            nc.gpsimd.iota(iota_r[:], pattern=[[1, P]], base=0, channel_multiplier=0)
            iota_rf = cp.tile([P, P], F32)
            nc.vector.tensor_copy(iota_rf[:], iota_r[:])
            iota_p = cp.tile([P, 1], I32)
            nc.gpsimd.iota(iota_p[:], pattern=[[1, 1]], base=0, channel_multiplier=1)
            iota_pf = cp.tile([P, 1], F32)
            nc.vector.tensor_copy(iota_pf[:], iota_p[:])
            ident = cp.tile([P, P], BF16)
            nc.vector.tensor_scalar(
                out=ident[:],
                in0=iota_rf[:],
                scalar1=iota_pf[:, 0:1],
                scalar2=None,
                op0=OP.is_equal,
            )
            # pad row: ft/er = 0, el = PAD_EL -> exp(leaky(lg)) == 0
            padt = cp.tile([1, RW], BF16)
            nc.gpsimd.memset(padt[:], 0.0)
            nc.gpsimd.memset(padt[:, 256:260], PAD_EL)
            nc.gpsimd.memset(padt[:, WC + 256 : WC + 260], PAD_EL)
            nc.sync.dma_start(out=g[NPOS : NPOS + 1, :], in_=padt[:])

            # ---- projection: g[q] = [ft|el|er](b0) | [ft|el|er](b1) ----
            hw = (nc.sync, nc.scalar)
            for b in range(B if stage in ('all', 'proj') else 0):
                for blk in range(NBLK):
                    j = b * NBLK + blk
                    ftb = pp.tile([DIN, QBLK], BF16, tag="ftb")
                    hw[j % 2].dma_start(
                        ftb[:],
                        featT[:, b * NPOS + blk * QBLK : b * NPOS + (blk + 1) * QBLK],
                    )
                    pw = pp.tile([P, QBLK // P, WC], BF16, tag="pw")
                    for i in range(QBLK // P):
                        po = ppp.tile([P, WC], F32, tag="po")
                        nc.tensor.matmul(
                            po[:],
                            lhsT=ftb[:, i * P : (i + 1) * P],
                            rhs=w_sb[:],
                            start=True,
                            stop=True,
                        )
                        if i % 2 == 0:
                            nc.vector.tensor_copy(pw[:, i, :], po[:])
                        else:
                            nc.scalar.activation(pw[:, i, :], po[:], AF.Copy)
                    dst_ap = g[
                        blk * QBLK : (blk + 1) * QBLK, b * WC : (b + 1) * WC
                    ].rearrange("(i p) c -> p i c", p=P)
                    hw[(j + 1) % 2].dma_start(out=dst_ap, in_=pw[:])

            # ---- aggregation: per tile, slot-aligned segment softmax+sum ----
            off8 = 0
            ncall = 0
            pending = []

            def _finalize(t, acc, ex, Kt):
                dsum = sm.tile([P, B * H], F32, tag="dsum")
                nc.vector.tensor_reduce(
                    out=dsum[:],
                    in_=ex[:, 0:Kt, :].rearrange("p k q -> p q k"),
                    axis=mybir.AxisListType.X,
                    op=OP.add,
                )
                dse = sm.tile([P, B * H], F32, tag="dse")
                nc.vector.tensor_scalar_add(dse[:], dsum[:], 1e-20)
                rcp = sm.tile([P, B * H, 1], F32, tag="rcp")
                nc.vector.reciprocal(rcp[:, :, 0], dse[:])
                o1 = sm.tile([P, B * H, D], F32, tag="o1")
                nc.vector.tensor_tensor(
                    out=o1[:],
                    in0=acc[:].rearrange("p (q d) -> p q d", d=D),
                    in1=rcp[:].to_broadcast([P, B * H, D]),
                    op=OP.mult,
                )
                og = sm.tile([P, FT], F32, tag="og")
                nc.vector.scalar_tensor_tensor(
                    out=og[:],
                    in0=o1[:].rearrange("p q d -> p (q d)"),
                    scalar=NEG,
                    in1=o1[:].rearrange("p q d -> p (q d)"),
                    op0=OP.mult,
                    op1=OP.max,
                )
                nc.sync.dma_start(out=outd[t * P : (t + 1) * P, :], in_=og[:])

            for t in range(NT if stage in ('all', 'agg') else 0):
                Kt = K[t]
                KP = Kt + 1
                gt = ag.tile([P, KP1M, RW], BF16, tag="gt")
                # SWDGE descriptor ring holds ~1024 pairs: cap 8 chunks/call;
                # rotate the SWDGE queues to overlap Q7 descriptor gen
                for g0 in range(0, KP, 8):
                    gn = min(8, KP - g0)
                    nc.gpsimd.dma_gather(
                        out_ap=gt[:, g0 : g0 + gn, :],
                        in_ap=g[:],
                        idxs_ap=idxs_res[:, off8 + g0 * 8 : off8 + (g0 + gn) * 8],
                        num_idxs=gn * P,
                        num_idxs_reg=gn * P,
                        elem_size=RW,
                        queue_num=ncall % NSWQ,
                    )
                    ncall += 1
                off8 += KP * 8

                # logits lg[p, k, (b,h)] = el_src + er_own (k-major: all the
                # downstream DVE/ACT ops stream contiguously)
                lg = sm.tile([P, KP1M, B * H], BF16, tag="lg")
                elv = gt[:, 0:Kt, 0 : 2 * WC].rearrange(
                    "p k (b r) -> p k b r", r=WC
                )[:, :, :, HD : HD + H]
                erv = gt[:, Kt, 0 : 2 * WC].rearrange("p (b r) -> p b r", r=WC)[
                    :, :, HD + H : HD + 2 * H
                ]
                lgv = lg[:, 0:Kt, :].rearrange("p k (b h) -> p k b h", h=H)
                nc.vector.tensor_tensor(
                    out=lgv,
                    in0=elv,
                    in1=erv[:, None, :, :].to_broadcast([P, Kt, B, H]),
                    op=OP.add,
                )
                l1 = sm.tile([P, KP1M, B * H], BF16, tag="l1")
                nc.vector.scalar_tensor_tensor(
                    out=l1[:, 0:Kt, :],
                    in0=lg[:, 0:Kt, :],
                    scalar=NEG,
                    in1=lg[:, 0:Kt, :],
                    op0=OP.mult,
                    op1=OP.max,
                )
                ex = sm.tile([P, KP1M, B * H], BF16, tag="ex")
                nc.scalar.activation(ex[:, 0:Kt, :], l1[:, 0:Kt, :], AF.Exp)

                # messages: ft *= ex (in-place, per batch)
                for b in range(B):
                    ftv = gt[:, 0:Kt, b * WC : b * WC + HD].rearrange(
                        "p k (h d) -> p k h d", d=D
                    )
                    exv = ex[:, 0:Kt, b * H : (b + 1) * H]
                    nc.vector.tensor_tensor(
                        out=ftv,
                        in0=ftv,
                        in1=exv[:, :, :, None].to_broadcast([P, Kt, H, D]),
                        op=OP.mult,
                    )

                # segment sum via identity matmuls accumulating in PSUM
                acc = psp.tile([P, FT], F32, tag="acc")
                for k in range(Kt):
                    rhs = gt[:, k, 0 : 2 * WC].rearrange("p (b r) -> p b r", r=WC)[
                        :, :, 0:HD
                    ]
                    nc.tensor.matmul(
                        acc[:],
                        lhsT=ident[:],
                        rhs=rhs,
                        start=(k == 0),
                        stop=(k == Kt - 1),
                    )

                pending.append((t, acc, ex, Kt))
                if len(pending) > 2:
                    _finalize(*pending.pop(0))
            for args in pending:
                _finalize(*args)

    nc.compile()
    return nc


def _make_inputs(feat, W, attn_l, attn_r, src, dst, n_nodes=N, n_cores=NCORES):
    feat = np.asarray(feat, dtype=np.float32)
    W = np.asarray(W, dtype=np.float32)
    attn_l = np.asarray(attn_l, dtype=np.float32)
    attn_r = np.asarray(attn_r, dtype=np.float32)
    src = np.asarray(src)
    dst = np.asarray(dst)

    K, idxs_all, pos2node = _host_prep(src, dst)

    real = pos2node >= 0
    ftp = np.zeros((B, NPOS, DIN), np.float32)
    ftp[:, real, :] = feat[pos2node[real]].transpose(1, 0, 2)
    featT = np.ascontiguousarray(ftp.reshape(B * NPOS, DIN).T).astype(
        ml_dtypes.bfloat16
    )
    Wl = (W.reshape(DIN, H, D) * attn_l[None]).sum(-1)
    Wr = (W.reshape(DIN, H, D) * attn_r[None]).sum(-1)
    wmat = np.concatenate([W, Wl, Wr], axis=1).astype(ml_dtypes.bfloat16)

    in_maps = [
        {"featT": featT, "wmat": wmat, "idxs": idxs_all[c]} for c in range(n_cores)
    ]
    return K, in_maps, pos2node


_CACHE = {}


def kernel(feat, W, attn_l, attn_r, src, dst):
    K, in_maps, pos2node = _make_inputs(feat, W, attn_l, attn_r, src, dst)
    key = tuple(K)
    if key not in _CACHE:
        _CACHE[key] = _build(K)
    nc = _CACHE[key]
    res = run_bass_kernel_spmd(nc, in_maps, list(range(NCORES))).results
    out = np.empty((N, B, H, D), np.float32)
    for c in range(NCORES):
        nodes = pos2node[c * NPOSC : c * NPOSC + NPC]
        out[nodes] = res[c]["out"][:NPC].reshape(NPC, B, H, D)
    return out


if __name__ == "__main__":
    rng = np.random.default_rng(0)
    feat = rng.standard_normal((N, B, DIN), dtype=np.float32)
    W = rng.standard_normal((DIN, H * D), dtype=np.float32) / np.sqrt(DIN)
    al = rng.standard_normal((H, D), dtype=np.float32) * 0.1
    ar = rng.standard_normal((H, D), dtype=np.float32) * 0.1
    src = rng.integers(0, N, E).astype(np.int32)
    dst = rng.integers(0, N, E).astype(np.int32)
    out = kernel(feat=feat, W=W, attn_l=al, attn_r=ar, src=src, dst=dst)
    print(out.shape, out.dtype, np.abs(out).mean())
